# revision 21
# baseline (speedup 1.0000x reference)
"""Trainium2 Bass kernel for CTC loss (nn_CTCLayer).

Inputs (full, unsharded):
  y_true       [64, 48]  int32  labels (blank excluded)
  y_pred       [64, 128, 4000] float32 probabilities
  label_length [64, 1]  int32
Output: loss [64, 1] float32  (= tf.keras ctc_batch_cost, input_length == T)

Strategy (pure data parallelism, 8 examples per core on 8 cores):

The CTC forward DP over S = 2L+1 = 97 extended states only ever touches
the <= L+1 classes in each example's extended label sequence, so the
HOST gathers those probability columns and pre-expands them into a
per-round coefficient tensor Q[state, round, chain] that the device
simply DMAs (no on-device gather / expansion / GPSIMD at all).

The DP runs in the scaled probability domain (q = kappa*(p + eps)) as
one stacked bidirectional chain of 63 rounds:

    X_r = (M^T X_{r-1}) * Q[:, r, :]      (PE matmul -> DVE multiply)

Columns 0:8 are the forward chains (fwd states on partitions 0..96),
columns 8:16 the backward chains stored PARTITION-FLIPPED (state s at
partition 96-s).  Under that flip the backward transition matrix equals
the forward one (J Bw J = F for the odd/even CTC band), so a single
static stationary matrix M drives both directions; per-example
repeated-label corrections use auxiliary rows 97..111 (fwd) and
112..127 (bwd), exactly cancelling the forbidden s-2 -> s transitions.

The meet at t* = 63: P(l|x) = sum_s (Band alpha_63)[s] * K_64[s].  The
final band application uses a second stationary matrix M2 that also
bakes in the partition flip, so the meet is a single masked
scalar_tensor_tensor against the flipped backward state, then a
ones-matmul column sum, Ln, and the exact log-domain corrections
(kappa bookkeeping plus two data-dependent renormalizations whose
factors are computed OFF the serial chain and folded into later Q
slots -- column scaling commutes with the per-column matmul).

Pathological inputs with more adjacent repeats than aux rows fall back
to an exact host computation (per core).
"""

import math
import os
import sys

import numpy as np

if "/opt/trn_rl_repo" not in sys.path:
    sys.path.insert(0, "/opt/trn_rl_repo")

# ---------------------------------------------------------------- constants
B, T, C, L = 64, 128, 4000, 48
S = 2 * L + 1            # 97 extended states
P = 128                  # partitions
NCORES = 8
BSH = B // NCORES        # 8 examples per core
BLANK = C - 1
EPS = 1e-7               # keras backend epsilon (reference adds before log)
KAPPA = 3328.0           # scale per q slot; exact bookkeeping at the end
NS = 64                  # Q slots: 0 = init (t=0 / t=127), 1..63 = rounds
NAUX = 15                # aux channels per chain (fwd 97..111, bwd 112..127)
RENREAD = (20, 41)       # rounds whose state column-sum feeds a renorm
RENAPP = (24, 45)        # rounds whose Q slot gets the 1/colsum factor
CQ = P                   # bfc column offsets: [M | Q | M2 | ones]
CM2 = CQ + NS * 16
CONES = CM2 + P
BFC_W = CONES + 1
FPC_W = 2 + P            # col0 = final mask, col1 spare, cols 2.. row0 ones
CORR = 128.0 * math.log(KAPPA)
LN2 = math.log(2.0)
# loss = CORR + sum_j [lnm_j + (e_j-127) ln2] - [lnm_f + (e_f-127) ln2]
#      = FCONST + sum_j lnq_j - lnm_f - e_f ln2,   lnq = lnm + e ln2
FCONST = CORR - (4 - 1) * 127.0 * LN2

_CACHE = {}


# ---------------------------------------------------------------- host tables
def _build_core_tables(y_true, y_pred, label_length):
    """y_true [8,L], y_pred [8,T,C], label_length [8] ->
    (bfc [128, BFC_W] bf16, fpc [128, FPC_W] f32, overflow: bool)."""
    import ml_dtypes

    n = y_true.shape[0]
    ll = label_length.reshape(-1).astype(np.int64)
    lab = np.where(np.arange(L)[None, :] < ll[:, None], y_true.astype(np.int64), BLANK)
    ext = np.full((n, S), BLANK, dtype=np.int64)
    ext[:, 1::2] = lab

    aug = []  # (i, b, s_i): repeat at odd state s_i (skip s_i-2 -> s_i forbidden)
    for b in range(n):
        for s_i in range(3, int(min(2 * ll[b] - 1, S - 1)) + 1, 2):
            j = (s_i - 1) // 2
            if lab[b, j] == lab[b, j - 1]:
                aug.append((len(aug), b, s_i))
    overflow = len(aug) > NAUX
    aug = aug[:NAUX]

    # forward band F (fwd state space): F[k, m] = allowed(k -> m), aux rows S+i
    F = np.zeros((P, P))
    for m in range(S):
        F[m, m] = 1.0
        if m >= 1:
            F[m - 1, m] = 1.0
        if m >= 2 and (m % 2 == 1):
            F[m - 2, m] = 1.0
    # backward band Bw: Bw[k, m] = allowed(m -> k); G' = Bw^T V
    Bw = np.zeros((S, S))
    for k in range(S):
        Bw[k, k] = 1.0
        if k >= 1:
            Bw[k, k - 1] = 1.0
        if k >= 2 and (k % 2 == 1):
            Bw[k, k - 2] = 1.0
    Bw_aux_rows = np.zeros((NAUX, S))   # bwd aux corrections in bwd state space
    for (i, b, s_i) in aug:
        Bw_aux_rows[i, s_i - 2] = -1.0

    for (i, b, s_i) in aug:        # aux rows into F before the col copies
        F[S + i, s_i] = -1.0

    flip = lambda s: 96 - s
    M = np.zeros((P, P))
    M[:S, :S] = F[:S, :S]          # == J Bw_core J (flip conjugation)
    for (i, b, s_i) in aug:        # fwd aux
        M[S + i, s_i] = -1.0
    for (i, b, s_i) in aug:
        M[:S, S + i] = F[:S, s_i - 2]
        for (i2, b2, s_i2) in aug:
            M[S + i2, S + i] = F[S + i2, s_i - 2]
    for (i, b, s_i) in aug:        # bwd aux (flipped embedding at rows 112+)
        M[112 + i, flip(s_i - 2)] = -1.0
    for (i, b, s_i) in aug:
        M[:S, 112 + i] = Bw[:S, s_i][::-1]
        for (i2, b2, s_i2) in aug:
            M[112 + i2, 112 + i] = Bw_aux_rows[i2, s_i]

    M2 = np.zeros((P, P))          # final band, output-flipped for the meet
    M2[:S, :S] = M[:S, :S][:, ::-1]
    for (i, b, s_i) in aug:
        M2[S + i, flip(s_i)] = -1.0

    # Q [128, NS, 16]
    Q = np.zeros((P, NS, 16), dtype=np.float32)
    for b in range(n):
        nlive = int(2 * ll[b] + 1)
        cls = ext[b]
        qf = KAPPA * (y_pred[b][:, cls].astype(np.float32) + EPS)   # [T, S]
        qf[:, nlive:] = 0.0
        Q[:S, :, b] = qf[0:NS, :].T
        Q[2:S, 0, b] = 0.0                         # fwd init: states 0,1 only
        qb = qf[:, ::-1]                           # flipped state axis
        Q[:S, :, 8 + b] = qb[127 - np.arange(NS), :].T
        em = np.zeros(S, dtype=np.float32)         # bwd init: end states
        em[96 - 2 * ll[b]] = 1.0
        em[96 - (2 * ll[b] - 1)] = 1.0
        Q[:S, 0, 8 + b] *= em
    for (i, b, s_i) in aug:
        j = (s_i - 1) // 2
        qf = KAPPA * (y_pred[b][:, lab[b, j - 1]].astype(np.float32) + EPS)  # [T]
        qb = KAPPA * (y_pred[b][:, lab[b, j]].astype(np.float32) + EPS)
        Q[S + i, :, b] = qf[0:NS]
        if s_i != 3:                               # aux tracks alpha[s_i-2]
            Q[S + i, 0, b] = 0.0
        Q[112 + i, :, 8 + b] = qb[127 - np.arange(NS)]
        if not (s_i == 2 * ll[b] or s_i == 2 * ll[b] - 1):
            Q[112 + i, 0, 8 + b] = 0.0

    bfc = np.zeros((P, BFC_W), dtype=ml_dtypes.bfloat16)
    bfc[:, 0:P] = M.astype(ml_dtypes.bfloat16)
    bfc[:, CQ:CM2] = Q.reshape(P, NS * 16).astype(ml_dtypes.bfloat16)
    bfc[:, CM2:CONES] = M2.astype(ml_dtypes.bfloat16)
    bfc[:, CONES] = ml_dtypes.bfloat16(1.0)

    fpc = np.zeros((P, FPC_W), dtype=np.float32)
    fpc[0:S, 0] = 1.0                              # final meet mask (unused)
    fpc[0, 1] = FCONST                             # kappa + exponent-bias const
    fpc[0, 2:2 + P] = 1.0                          # ones row (bc matmul lhsT)
    return bfc, fpc, overflow


# ---------------------------------------------------------------- host fallback
def _host_ctc(y_true_b, y_pred_b, ll_b):
    """Exact log-domain port of the reference for one example (float64)."""
    NEG = -1e30
    ll = int(ll_b)
    lab = np.where(np.arange(L) < ll, y_true_b.astype(np.int64), BLANK)
    ext = np.full((S,), BLANK, dtype=np.int64)
    ext[1::2] = lab
    lp = np.log(y_pred_b.astype(np.float64) + EPS)[:, ext]    # [T, S]
    ext_m2 = np.concatenate([[BLANK, BLANK], ext[:-2]])
    allow = (ext != BLANK) & (ext != ext_m2)
    alpha = np.where(np.arange(S) < 2, lp[0], NEG)
    for t in range(1, T):
        a0 = alpha
        a1 = np.concatenate([[NEG], alpha[:-1]])
        a2 = np.where(allow, np.concatenate([[NEG, NEG], alpha[:-2]]), NEG)
        m = np.maximum(np.maximum(a0, a1), a2)
        alpha = m + np.log(np.exp(a0 - m) + np.exp(a1 - m) + np.exp(a2 - m)) + lp[t]
    ab, al = alpha[2 * ll], alpha[2 * ll - 1]
    m = max(ab, al)
    return -(m + math.log(math.exp(ab - m) + math.exp(al - m)))


# ---------------------------------------------------------------- bass program
def _build_program():
    import concourse.bacc as bacc
    import concourse.tile as tile
    import concourse.mybir as mybir

    debug = bool(int(os.environ.get("CTC_DEBUG", "0")))
    nc = bacc.Bacc("TRN2", target_bir_lowering=False, debug=False,
                   enable_asserts=False, num_devices=NCORES, num_swdge_queues=4)
    bfc_d = nc.dram_tensor("bfc", [P, BFC_W], mybir.dt.bfloat16, kind="ExternalInput")
    fpc_d = nc.dram_tensor("fpc", [P, FPC_W], mybir.dt.float32, kind="ExternalInput")
    OW = 64 if debug else BSH
    loss_d = nc.dram_tensor("loss", [1, OW], mybir.dt.float32, kind="ExternalOutput")

    fp32 = mybir.dt.float32
    bf16 = mybir.dt.bfloat16
    mult = mybir.AluOpType.mult
    add = mybir.AluOpType.add

    with tile.TileContext(nc) as tc:
        with (
            tc.tile_pool(name="cpool", bufs=1) as cpool,
            tc.tile_pool(name="upool", bufs=3) as upool,
            tc.tile_pool(name="spool", bufs=1) as spool,
            tc.tile_pool(name="psx", bufs=2, space="PSUM") as psx,
            tc.tile_pool(name="pss", bufs=1, space="PSUM") as pss,
        ):
            bfc = cpool.tile([P, BFC_W], bf16, tag="bfc")
            # M halves land first on two parallel queues, then the first Q
            # slots, then the bulk; compute starts as soon as M + slot 0 land.
            nc.sync.dma_start(bfc[:, 0:64], bfc_d[:, 0:64])
            nc.scalar.dma_start(bfc[:, 64:P], bfc_d[:, 64:P])
            nc.sync.dma_start(bfc[:, CQ:CQ + 32], bfc_d[:, CQ:CQ + 32])
            nc.scalar.dma_start(bfc[:, CQ + 32:CQ + 128], bfc_d[:, CQ + 32:CQ + 128])
            nc.sync.dma_start(bfc[:, CQ + 128:BFC_W], bfc_d[:, CQ + 128:BFC_W])
            fpc = cpool.tile([P, FPC_W], fp32, tag="fpc")
            nc.scalar.dma_start(fpc[:], fpc_d[:])

            M_ap = bfc[:, 0:P]
            M2_ap = bfc[:, CM2:CONES]
            ones_ap = bfc[:, CONES:CONES + 1]
            Qs = lambda r: bfc[:, CQ + 16 * r:CQ + 16 * (r + 1)]

            norms = spool.tile([1, 2 * 16], fp32, tag="norms")
            qsc0 = spool.tile([P, 16], fp32, tag="qsc0")
            qsc1 = spool.tile([P, 16], fp32, tag="qsc1")
            qsc = {RENAPP[0]: qsc0, RENAPP[1]: qsc1}

            X = None
            H = BSH
            for r in range(1, NS):
                # fwd (cols 0:8) and bwd (cols 8:16) as separate MM+TT pairs:
                # the chains decouple and phase-offset by half a round, and
                # each dependent hop is the cheaper 8-wide op.
                ps = psx.tile([P, 16], fp32, tag="ps")
                rhs = Qs(0) if X is None else X[:]
                nc.tensor.matmul(ps[:, 0:H], M_ap, rhs[:, 0:H],
                                 start=True, stop=True)
                nc.tensor.matmul(ps[:, H:2 * H], M_ap, rhs[:, H:2 * H],
                                 start=True, stop=True)
                Xn = upool.tile([P, 16], bf16, tag="X")
                in1 = qsc[r][:] if r in RENAPP else Qs(r)
                nc.vector.tensor_tensor(out=Xn[:, 0:H], in0=ps[:, 0:H],
                                        in1=in1[:, 0:H], op=mult)
                nc.vector.tensor_tensor(out=Xn[:, H:2 * H], in0=ps[:, H:2 * H],
                                        in1=in1[:, H:2 * H], op=mult)
                X = Xn
                if r in RENREAD:
                    # off the serial chain: colsum of X_r scales Q slot r+4;
                    # column scaling commutes with the per-column matmul, and
                    # the exact reciprocal applied is logged for the end.
                    k = RENREAD.index(r)
                    nm = pss.tile([1, 16], fp32, tag="nm")
                    nc.tensor.matmul(nm[:], ones_ap, X[:], start=True, stop=True)
                    rrow = norms[0:1, k * 16:(k + 1) * 16]
                    nc.vector.reciprocal(rrow, nm[:])
                    bc = pss.tile([P, 16], fp32, tag="bc")
                    nc.tensor.matmul(bc[:], fpc[0:1, 2:2 + P], rrow,
                                     start=True, stop=True)
                    nc.vector.tensor_tensor(
                        out=qsc[RENAPP[k]][:], in0=Qs(RENAPP[k]), in1=bc[:], op=mult)

            # meet: fin[b] = sum_s (Band alpha_63)[s] * K_64[s]; M2 bakes the
            # partition flip so both operands align and its zero aux columns
            # already blank partitions 97..127 of ps64.
            ps64 = pss.tile([P, BSH], fp32, tag="ps64")
            nc.tensor.matmul(ps64[:], M2_ap, X[:, 0:BSH], start=True, stop=True)
            prod = spool.tile([P, BSH], bf16, tag="prod")
            nc.vector.tensor_tensor(
                out=prod[:], in0=ps64[:], in1=X[:, BSH:2 * BSH], op=mult)
            fin = pss.tile([1, BSH], fp32, tag="fin")
            nc.tensor.matmul(fin[:], ones_ap, prod[:], start=True, stop=True)

            # Exact full-range ln: the ACT Ln table loses absolute accuracy for
            # inputs far from 1 (catastrophically below ~2^-64), so split off
            # the exponent with integer ops and Ln only the mantissa in [1,2).
            i32 = mybir.dt.int32
            shr = mybir.AluOpType.logical_shift_right
            band = mybir.AluOpType.bitwise_and
            bor = mybir.AluOpType.bitwise_or
            Ln = mybir.ActivationFunctionType.Ln

            # off-chain: ln of the renorm reciprocals via the same split
            nm = spool.tile([1, 2 * 16], i32, tag="nm")
            nc.vector.tensor_scalar(nm[:], norms[:].bitcast(i32),
                                    0x007FFFFF, 0x3F800000, band, bor)
            ne = spool.tile([1, 2 * 16], i32, tag="ne")
            nc.vector.tensor_scalar(ne[:], norms[:].bitcast(i32), 23, None, shr)
            nef = spool.tile([1, 2 * 16], fp32, tag="nef")
            nc.vector.tensor_copy(nef[:], ne[:])
            nlnm = spool.tile([1, 2 * 16], fp32, tag="nlnm")
            nc.scalar.activation(nlnm[:], nm[:].bitcast(fp32), Ln)
            lnq = spool.tile([1, 2 * 16], fp32, tag="lnq")
            nc.vector.scalar_tensor_tensor(
                out=lnq[:], in0=nef[:], scalar=LN2, in1=nlnm[:], op0=mult, op1=add)
            lnrsum = spool.tile([1, BSH], fp32, tag="lnrsum")
            nc.vector.reduce_sum(
                lnrsum[:],
                lnq[0:1, :].rearrange("p (j b) -> p b j", j=4),
                axis=mybir.AxisListType.X)
            v = spool.tile([1, BSH], fp32, tag="v")
            nc.vector.tensor_scalar_add(v[:], lnrsum[:], fpc[0:1, 1:2])

            # tail: same split for fin (mantissa first so the Ln starts sooner)
            fm = spool.tile([1, BSH], i32, tag="fm")
            nc.vector.tensor_scalar(fm[:], fin[:].bitcast(i32),
                                    0x007FFFFF, 0x3F800000, band, bor)
            fe = spool.tile([1, BSH], i32, tag="fe")
            nc.vector.tensor_scalar(fe[:], fin[:].bitcast(i32), 23, None, shr)
            fef = spool.tile([1, BSH], fp32, tag="fef")
            nc.vector.tensor_copy(fef[:], fe[:])
            flnm = spool.tile([1, BSH], fp32, tag="flnm")
            nc.scalar.activation(flnm[:], fm[:].bitcast(fp32), Ln)
            t1 = spool.tile([1, BSH], fp32, tag="t1")
            nc.vector.scalar_tensor_tensor(
                out=t1[:], in0=fef[:], scalar=-LN2, in1=v[:], op0=mult, op1=add)
            loss_row = spool.tile([1, OW], fp32, tag="loss_row")
            nc.vector.scalar_tensor_tensor(
                out=loss_row[0:1, 0:BSH], in0=flnm[:], scalar=-1.0, in1=t1[:],
                op0=mult, op1=add)
            if debug:
                nc.vector.tensor_scalar_add(loss_row[0:1, 8:16], fin[:], 0.0)
                nc.vector.tensor_scalar_add(loss_row[0:1, 16:24], flnm[:], 0.0)
                nc.vector.tensor_scalar_add(loss_row[0:1, 24:32], fef[:], 0.0)
                nc.vector.tensor_scalar_add(loss_row[0:1, 32:64], norms[:], 0.0)
            nc.sync.dma_start(loss_d[:], loss_row[:])

    nc.compile()
    return nc


def _get_program():
    if "nc" not in _CACHE:
        _CACHE["nc"] = _build_program()
    return _CACHE["nc"]


# ---------------------------------------------------------------- entry point
def kernel(y_true: np.ndarray, y_pred: np.ndarray, label_length: np.ndarray) -> np.ndarray:
    from concourse.bass_utils import run_bass_kernel_spmd

    y_true = np.asarray(y_true)
    y_pred = np.asarray(y_pred, dtype=np.float32)
    label_length = np.asarray(label_length)
    assert y_true.shape == (B, L) and y_pred.shape == (B, T, C), (
        f"unexpected shapes {y_true.shape} {y_pred.shape}")

    ll_all = label_length.reshape(-1)
    in_maps = []
    fallback_cores = []
    for core in range(NCORES):
        sl = slice(core * BSH, (core + 1) * BSH)
        bfc, fpc, overflow = _build_core_tables(y_true[sl], y_pred[sl], ll_all[sl])
        if overflow:
            fallback_cores.append(core)
        in_maps.append({"bfc": bfc, "fpc": fpc})

    nc = _get_program()
    res = run_bass_kernel_spmd(
        nc, in_maps, core_ids=list(range(NCORES)),
        trace=bool(int(os.environ.get("CTC_TRACE", "0"))),
    )
    _CACHE["last_result"] = res

    loss = np.zeros((B, 1), dtype=np.float32)
    _CACHE["debug_rows"] = [res.results[c]["loss"][0] for c in range(NCORES)]
    for core in range(NCORES):
        loss[core * BSH:(core + 1) * BSH, 0] = res.results[core]["loss"][0][:BSH]

    for core in fallback_cores:  # more repeats than aux rows (pathological)
        for b in range(BSH):
            g = core * BSH + b
            loss[g, 0] = _host_ctc(y_true[g], y_pred[g], ll_all[g])
    return loss


# revision 23
# speedup vs baseline: 1.2919x; 1.2919x over previous
"""Trainium2 Bass kernel for CTC loss (nn_CTCLayer).

Inputs (full, unsharded):
  y_true       [64, 48]  int32  labels (blank excluded)
  y_pred       [64, 128, 4000] float32 probabilities
  label_length [64, 1]  int32
Output: loss [64, 1] float32  (= tf.keras ctc_batch_cost, input_length == T)

Strategy (pure data parallelism, 8 examples per core on 8 cores):

The CTC forward DP over S = 2L+1 = 97 extended states only ever touches
the <= L+1 classes in each example's extended label sequence, so the
HOST gathers those probability columns and pre-expands them into a
per-round coefficient tensor Q[state, round, chain] that the device
simply DMAs (no on-device gather / expansion / GPSIMD at all).

The DP runs in the scaled probability domain (q = kappa*(p + eps)) as
one stacked bidirectional chain of 63 rounds:

    X_r = (M^T X_{r-1}) * Q[:, r, :]      (PE matmul -> DVE multiply)

Columns 0:8 are the forward chains (fwd states on partitions 0..96),
columns 8:16 the backward chains stored PARTITION-FLIPPED (state s at
partition 96-s).  Under that flip the backward transition matrix equals
the forward one (J Bw J = F for the odd/even CTC band), so a single
static stationary matrix M drives both directions; per-example
repeated-label corrections use auxiliary rows 97..111 (fwd) and
112..127 (bwd), exactly cancelling the forbidden s-2 -> s transitions.

The meet at t* = 63: P(l|x) = sum_s (Band alpha_63)[s] * K_64[s].  The
final band application uses a second stationary matrix M2 that also
bakes in the partition flip, so the meet is a single masked
scalar_tensor_tensor against the flipped backward state, then a
ones-matmul column sum, Ln, and the exact log-domain corrections
(kappa bookkeeping plus two data-dependent renormalizations whose
factors are computed OFF the serial chain and folded into later Q
slots -- column scaling commutes with the per-column matmul).

Pathological inputs with more adjacent repeats than aux rows fall back
to an exact host computation (per core).
"""

import math
import os
import sys

import numpy as np

if "/opt/trn_rl_repo" not in sys.path:
    sys.path.insert(0, "/opt/trn_rl_repo")

# ---------------------------------------------------------------- constants
B, T, C, L = 64, 128, 4000, 48
S = 2 * L + 1            # 97 extended states
P = 128                  # partitions
NCORES = 8
BSH = B // NCORES        # 8 examples per core
BLANK = C - 1
EPS = 1e-7               # keras backend epsilon (reference adds before log)
KAPPA = 3328.0           # scale per q slot; exact bookkeeping at the end
NS = 64                  # Q slots: 0 = init (t=0 / t=127), 1..63 = rounds
NAUX = 15                # aux channels per chain (fwd 97..111, bwd 112..127)
RENREAD = (20, 41)       # rounds whose state column-sum feeds a renorm
RENAPP = (24, 45)        # rounds whose Q slot gets the 1/colsum factor
CQ = P                   # bfc column offsets: [M | Q | M2 | ones]
CM2 = CQ + NS * 16
CONES = CM2 + P
BFC_W = CONES + 1
FPC_W = 2 + P            # col0 = final mask, col1 spare, cols 2.. row0 ones
CORR = 128.0 * math.log(KAPPA)
LN2 = math.log(2.0)
# loss = CORR + sum_j [lnm_j + (e_j-127) ln2] - [lnm_f + (e_f-127) ln2]
#      = FCONST + sum_j lnq_j - lnm_f - e_f ln2,   lnq = lnm + e ln2
FCONST = CORR - (4 - 1) * 127.0 * LN2

_CACHE = {}


# ---------------------------------------------------------------- host tables
def _build_core_tables(y_true, y_pred, label_length):
    """y_true [8,L], y_pred [8,T,C], label_length [8] ->
    (bfc [128, BFC_W] bf16, fpc [128, FPC_W] f32, overflow: bool)."""
    import ml_dtypes

    n = y_true.shape[0]
    ll = label_length.reshape(-1).astype(np.int64)
    lab = np.where(np.arange(L)[None, :] < ll[:, None], y_true.astype(np.int64), BLANK)
    ext = np.full((n, S), BLANK, dtype=np.int64)
    ext[:, 1::2] = lab

    aug = []  # (i, b, s_i): repeat at odd state s_i (skip s_i-2 -> s_i forbidden)
    for b in range(n):
        for s_i in range(3, int(min(2 * ll[b] - 1, S - 1)) + 1, 2):
            j = (s_i - 1) // 2
            if lab[b, j] == lab[b, j - 1]:
                aug.append((len(aug), b, s_i))
    overflow = len(aug) > NAUX
    aug = aug[:NAUX]

    # forward band F (fwd state space): F[k, m] = allowed(k -> m), aux rows S+i
    F = np.zeros((P, P))
    for m in range(S):
        F[m, m] = 1.0
        if m >= 1:
            F[m - 1, m] = 1.0
        if m >= 2 and (m % 2 == 1):
            F[m - 2, m] = 1.0
    # backward band Bw: Bw[k, m] = allowed(m -> k); G' = Bw^T V
    Bw = np.zeros((S, S))
    for k in range(S):
        Bw[k, k] = 1.0
        if k >= 1:
            Bw[k, k - 1] = 1.0
        if k >= 2 and (k % 2 == 1):
            Bw[k, k - 2] = 1.0
    Bw_aux_rows = np.zeros((NAUX, S))   # bwd aux corrections in bwd state space
    for (i, b, s_i) in aug:
        Bw_aux_rows[i, s_i - 2] = -1.0

    for (i, b, s_i) in aug:        # aux rows into F before the col copies
        F[S + i, s_i] = -1.0

    flip = lambda s: 96 - s
    M = np.zeros((P, P))
    M[:S, :S] = F[:S, :S]          # == J Bw_core J (flip conjugation)
    for (i, b, s_i) in aug:        # fwd aux
        M[S + i, s_i] = -1.0
    for (i, b, s_i) in aug:
        M[:S, S + i] = F[:S, s_i - 2]
        for (i2, b2, s_i2) in aug:
            M[S + i2, S + i] = F[S + i2, s_i - 2]
    for (i, b, s_i) in aug:        # bwd aux (flipped embedding at rows 112+)
        M[112 + i, flip(s_i - 2)] = -1.0
    for (i, b, s_i) in aug:
        M[:S, 112 + i] = Bw[:S, s_i][::-1]
        for (i2, b2, s_i2) in aug:
            M[112 + i2, 112 + i] = Bw_aux_rows[i2, s_i]

    M2 = np.zeros((P, P))          # final band, output-flipped for the meet
    M2[:S, :S] = M[:S, :S][:, ::-1]
    for (i, b, s_i) in aug:
        M2[S + i, flip(s_i)] = -1.0

    # Q [128, NS, 16]
    Q = np.zeros((P, NS, 16), dtype=np.float32)
    for b in range(n):
        nlive = int(2 * ll[b] + 1)
        cls = ext[b]
        qf = KAPPA * (y_pred[b][:, cls].astype(np.float32) + EPS)   # [T, S]
        qf[:, nlive:] = 0.0
        Q[:S, :, b] = qf[0:NS, :].T
        Q[2:S, 0, b] = 0.0                         # fwd init: states 0,1 only
        qb = qf[:, ::-1]                           # flipped state axis
        Q[:S, :, 8 + b] = qb[127 - np.arange(NS), :].T
        em = np.zeros(S, dtype=np.float32)         # bwd init: end states
        em[96 - 2 * ll[b]] = 1.0
        em[96 - (2 * ll[b] - 1)] = 1.0
        Q[:S, 0, 8 + b] *= em
    for (i, b, s_i) in aug:
        j = (s_i - 1) // 2
        qf = KAPPA * (y_pred[b][:, lab[b, j - 1]].astype(np.float32) + EPS)  # [T]
        qb = KAPPA * (y_pred[b][:, lab[b, j]].astype(np.float32) + EPS)
        Q[S + i, :, b] = qf[0:NS]
        if s_i != 3:                               # aux tracks alpha[s_i-2]
            Q[S + i, 0, b] = 0.0
        Q[112 + i, :, 8 + b] = qb[127 - np.arange(NS)]
        if not (s_i == 2 * ll[b] or s_i == 2 * ll[b] - 1):
            Q[112 + i, 0, 8 + b] = 0.0

    bfc = np.zeros((P, BFC_W), dtype=ml_dtypes.bfloat16)
    bfc[:, 0:P] = M.astype(ml_dtypes.bfloat16)
    bfc[:, CQ:CM2] = Q.reshape(P, NS * 16).astype(ml_dtypes.bfloat16)
    bfc[:, CM2:CONES] = M2.astype(ml_dtypes.bfloat16)
    bfc[:, CONES] = ml_dtypes.bfloat16(1.0)

    fpc = np.zeros((P, FPC_W), dtype=np.float32)
    fpc[0:S, 0] = 1.0                              # final meet mask (unused)
    fpc[0, 1] = FCONST                             # kappa + exponent-bias const
    fpc[0, 2:2 + P] = 1.0                          # ones row (bc matmul lhsT)
    return bfc, fpc, overflow


# ---------------------------------------------------------------- host fallback
def _host_ctc(y_true_b, y_pred_b, ll_b):
    """Exact log-domain port of the reference for one example (float64)."""
    NEG = -1e30
    ll = int(ll_b)
    lab = np.where(np.arange(L) < ll, y_true_b.astype(np.int64), BLANK)
    ext = np.full((S,), BLANK, dtype=np.int64)
    ext[1::2] = lab
    lp = np.log(y_pred_b.astype(np.float64) + EPS)[:, ext]    # [T, S]
    ext_m2 = np.concatenate([[BLANK, BLANK], ext[:-2]])
    allow = (ext != BLANK) & (ext != ext_m2)
    alpha = np.where(np.arange(S) < 2, lp[0], NEG)
    for t in range(1, T):
        a0 = alpha
        a1 = np.concatenate([[NEG], alpha[:-1]])
        a2 = np.where(allow, np.concatenate([[NEG, NEG], alpha[:-2]]), NEG)
        m = np.maximum(np.maximum(a0, a1), a2)
        alpha = m + np.log(np.exp(a0 - m) + np.exp(a1 - m) + np.exp(a2 - m)) + lp[t]
    ab, al = alpha[2 * ll], alpha[2 * ll - 1]
    m = max(ab, al)
    return -(m + math.log(math.exp(ab - m) + math.exp(al - m)))


# ---------------------------------------------------------------- bass program
def _build_program():
    import concourse.bacc as bacc
    import concourse.tile as tile
    import concourse.mybir as mybir

    debug = bool(int(os.environ.get("CTC_DEBUG", "0")))
    nc = bacc.Bacc("TRN2", target_bir_lowering=False, debug=False,
                   enable_asserts=False, num_devices=NCORES, num_swdge_queues=4)
    bfc_d = nc.dram_tensor("bfc", [P, BFC_W], mybir.dt.bfloat16, kind="ExternalInput")
    fpc_d = nc.dram_tensor("fpc", [P, FPC_W], mybir.dt.float32, kind="ExternalInput")
    OW = 64 if debug else BSH
    loss_d = nc.dram_tensor("loss", [1, OW], mybir.dt.float32, kind="ExternalOutput")

    fp32 = mybir.dt.float32
    bf16 = mybir.dt.bfloat16
    mult = mybir.AluOpType.mult
    add = mybir.AluOpType.add

    with tile.TileContext(nc) as tc:
        with (
            tc.tile_pool(name="cpool", bufs=1) as cpool,
            tc.tile_pool(name="upool", bufs=3) as upool,
            tc.tile_pool(name="spool", bufs=1) as spool,
            tc.tile_pool(name="psx", bufs=2, space="PSUM") as psx,
            tc.tile_pool(name="pss", bufs=1, space="PSUM") as pss,
        ):
            bfc = cpool.tile([P, BFC_W], bf16, tag="bfc")
            # M halves land first on two parallel queues, then the first Q
            # slots, then the bulk; compute starts as soon as M + slot 0 land.
            nc.sync.dma_start(bfc[:, 0:64], bfc_d[:, 0:64])
            nc.scalar.dma_start(bfc[:, 64:P], bfc_d[:, 64:P])
            nc.sync.dma_start(bfc[:, CQ:CQ + 32], bfc_d[:, CQ:CQ + 32])
            nc.scalar.dma_start(bfc[:, CQ + 32:CQ + 128], bfc_d[:, CQ + 32:CQ + 128])
            nc.sync.dma_start(bfc[:, CQ + 128:BFC_W], bfc_d[:, CQ + 128:BFC_W])
            fpc = cpool.tile([P, FPC_W], fp32, tag="fpc")
            nc.scalar.dma_start(fpc[:], fpc_d[:])

            M_ap = bfc[:, 0:P]
            M2_ap = bfc[:, CM2:CONES]
            ones_ap = bfc[:, CONES:CONES + 1]
            Qs = lambda r: bfc[:, CQ + 16 * r:CQ + 16 * (r + 1)]

            norms = spool.tile([1, 2 * 16], fp32, tag="norms")
            qsc0 = spool.tile([P, 16], fp32, tag="qsc0")
            qsc1 = spool.tile([P, 16], fp32, tag="qsc1")
            qsc = {RENAPP[0]: qsc0, RENAPP[1]: qsc1}

            Xf = None
            Xb = None
            H = BSH
            for r in range(1, NS):
                # fwd (cols 0:8) and bwd (cols 8:16) as fully separate MM+TT
                # pairs with their own PSUM/SBUF tiles: the chains decouple
                # and phase-offset by half a round (PSUM deps are tracked at
                # tile granularity, so sharing a psum tile would serialize).
                psf = psx.tile([P, H], fp32, tag="psf")
                nc.tensor.matmul(psf[:], M_ap,
                                 Qs(0)[:, 0:H] if Xf is None else Xf[:],
                                 start=True, stop=True)
                psb = psx.tile([P, H], fp32, tag="psb")
                nc.tensor.matmul(psb[:], M_ap,
                                 Qs(0)[:, H:2 * H] if Xb is None else Xb[:],
                                 start=True, stop=True)
                in1 = qsc[r][:] if r in RENAPP else Qs(r)
                Xfn = upool.tile([P, H], bf16, tag="Xf")
                nc.vector.tensor_tensor(out=Xfn[:], in0=psf[:],
                                        in1=in1[:, 0:H], op=mult)
                Xbn = upool.tile([P, H], bf16, tag="Xb")
                nc.vector.tensor_tensor(out=Xbn[:], in0=psb[:],
                                        in1=in1[:, H:2 * H], op=mult)
                Xf, Xb = Xfn, Xbn
                if r in RENREAD:
                    # off the serial chain: colsum of X_r scales Q slot r+4;
                    # column scaling commutes with the per-column matmul, and
                    # the exact reciprocal applied is logged for the end.
                    k = RENREAD.index(r)
                    nm = pss.tile([1, 16], fp32, tag="nm")
                    nc.tensor.matmul(nm[0:1, 0:H], ones_ap, Xf[:],
                                     start=True, stop=True)
                    nc.tensor.matmul(nm[0:1, H:2 * H], ones_ap, Xb[:],
                                     start=True, stop=True)
                    rrow = norms[0:1, k * 16:(k + 1) * 16]
                    nc.vector.reciprocal(rrow, nm[:])
                    bc = pss.tile([P, 16], fp32, tag="bc")
                    nc.tensor.matmul(bc[:], fpc[0:1, 2:2 + P], rrow,
                                     start=True, stop=True)
                    nc.vector.tensor_tensor(
                        out=qsc[RENAPP[k]][:], in0=Qs(RENAPP[k]), in1=bc[:], op=mult)

            # meet: fin[b] = sum_s (Band alpha_63)[s] * K_64[s]; M2 bakes the
            # partition flip so both operands align and its zero aux columns
            # already blank partitions 97..127 of ps64.
            ps64 = pss.tile([P, BSH], fp32, tag="ps64")
            nc.tensor.matmul(ps64[:], M2_ap, Xf[:], start=True, stop=True)
            prod = spool.tile([P, BSH], bf16, tag="prod")
            nc.vector.tensor_tensor(
                out=prod[:], in0=ps64[:], in1=Xb[:], op=mult)
            fin = pss.tile([1, BSH], fp32, tag="fin")
            nc.tensor.matmul(fin[:], ones_ap, prod[:], start=True, stop=True)

            # Exact full-range ln: the ACT Ln table loses absolute accuracy for
            # inputs far from 1 (catastrophically below ~2^-64), so split off
            # the exponent with integer ops and Ln only the mantissa in [1,2).
            i32 = mybir.dt.int32
            shr = mybir.AluOpType.logical_shift_right
            band = mybir.AluOpType.bitwise_and
            bor = mybir.AluOpType.bitwise_or
            Ln = mybir.ActivationFunctionType.Ln

            # off-chain: ln of the renorm reciprocals via the same split
            nm = spool.tile([1, 2 * 16], i32, tag="nm")
            nc.vector.tensor_scalar(nm[:], norms[:].bitcast(i32),
                                    0x007FFFFF, 0x3F800000, band, bor)
            ne = spool.tile([1, 2 * 16], i32, tag="ne")
            nc.vector.tensor_scalar(ne[:], norms[:].bitcast(i32), 23, None, shr)
            nef = spool.tile([1, 2 * 16], fp32, tag="nef")
            nc.vector.tensor_copy(nef[:], ne[:])
            nlnm = spool.tile([1, 2 * 16], fp32, tag="nlnm")
            nc.scalar.activation(nlnm[:], nm[:].bitcast(fp32), Ln)
            lnq = spool.tile([1, 2 * 16], fp32, tag="lnq")
            nc.vector.scalar_tensor_tensor(
                out=lnq[:], in0=nef[:], scalar=LN2, in1=nlnm[:], op0=mult, op1=add)
            lnrsum = spool.tile([1, BSH], fp32, tag="lnrsum")
            nc.vector.reduce_sum(
                lnrsum[:],
                lnq[0:1, :].rearrange("p (j b) -> p b j", j=4),
                axis=mybir.AxisListType.X)
            v = spool.tile([1, BSH], fp32, tag="v")
            nc.vector.tensor_scalar_add(v[:], lnrsum[:], fpc[0:1, 1:2])

            # tail: same split for fin (mantissa first so the Ln starts sooner)
            fm = spool.tile([1, BSH], i32, tag="fm")
            nc.vector.tensor_scalar(fm[:], fin[:].bitcast(i32),
                                    0x007FFFFF, 0x3F800000, band, bor)
            fe = spool.tile([1, BSH], i32, tag="fe")
            nc.vector.tensor_scalar(fe[:], fin[:].bitcast(i32), 23, None, shr)
            fef = spool.tile([1, BSH], fp32, tag="fef")
            nc.vector.tensor_copy(fef[:], fe[:])
            flnm = spool.tile([1, BSH], fp32, tag="flnm")
            nc.scalar.activation(flnm[:], fm[:].bitcast(fp32), Ln)
            t1 = spool.tile([1, BSH], fp32, tag="t1")
            nc.vector.scalar_tensor_tensor(
                out=t1[:], in0=fef[:], scalar=-LN2, in1=v[:], op0=mult, op1=add)
            loss_row = spool.tile([1, OW], fp32, tag="loss_row")
            nc.vector.scalar_tensor_tensor(
                out=loss_row[0:1, 0:BSH], in0=flnm[:], scalar=-1.0, in1=t1[:],
                op0=mult, op1=add)
            if debug:
                nc.vector.tensor_scalar_add(loss_row[0:1, 8:16], fin[:], 0.0)
                nc.vector.tensor_scalar_add(loss_row[0:1, 16:24], flnm[:], 0.0)
                nc.vector.tensor_scalar_add(loss_row[0:1, 24:32], fef[:], 0.0)
                nc.vector.tensor_scalar_add(loss_row[0:1, 32:64], norms[:], 0.0)
            nc.sync.dma_start(loss_d[:], loss_row[:])

    nc.compile()
    return nc


def _get_program():
    if "nc" not in _CACHE:
        _CACHE["nc"] = _build_program()
    return _CACHE["nc"]


# ---------------------------------------------------------------- entry point
def kernel(y_true: np.ndarray, y_pred: np.ndarray, label_length: np.ndarray) -> np.ndarray:
    from concourse.bass_utils import run_bass_kernel_spmd

    y_true = np.asarray(y_true)
    y_pred = np.asarray(y_pred, dtype=np.float32)
    label_length = np.asarray(label_length)
    assert y_true.shape == (B, L) and y_pred.shape == (B, T, C), (
        f"unexpected shapes {y_true.shape} {y_pred.shape}")

    ll_all = label_length.reshape(-1)
    in_maps = []
    fallback_cores = []
    for core in range(NCORES):
        sl = slice(core * BSH, (core + 1) * BSH)
        bfc, fpc, overflow = _build_core_tables(y_true[sl], y_pred[sl], ll_all[sl])
        if overflow:
            fallback_cores.append(core)
        in_maps.append({"bfc": bfc, "fpc": fpc})

    nc = _get_program()
    res = run_bass_kernel_spmd(
        nc, in_maps, core_ids=list(range(NCORES)),
        trace=bool(int(os.environ.get("CTC_TRACE", "0"))),
    )
    _CACHE["last_result"] = res

    loss = np.zeros((B, 1), dtype=np.float32)
    _CACHE["debug_rows"] = [res.results[c]["loss"][0] for c in range(NCORES)]
    for core in range(NCORES):
        loss[core * BSH:(core + 1) * BSH, 0] = res.results[core]["loss"][0][:BSH]

    for core in fallback_cores:  # more repeats than aux rows (pathological)
        for b in range(BSH):
            g = core * BSH + b
            loss[g, 0] = _host_ctc(y_true[g], y_pred[g], ll_all[g])
    return loss
